# revision 4
# baseline (speedup 1.0000x reference)
"""STFT (DFT-as-conv) kernel for Trainium2, 8 NeuronCores.

Problem: x (16, 262144) f32, hann-windowed DFT kernels wsin/wcos
(2048, 1, 2048); reference reflect-pads by 1024, convolves with hop 512
-> returns (real, -imag), each (16, 2048, 513) f32.

Strategy:
  - Host: reflect-pad x; build per-batch block-transpose ("im2col by
    hop blocks"): bt[b, c, jj, m] = xpad[b, 512*m + 128*c + jj].
    Since n_fft = 4*hop, the frame matrix is 4 shifted views of bt.
  - Spectral symmetry: bins k and 2048-k mirror (cos even, sin odd).
    Device computes bins 0..1151 (9 chunks of 128); host mirrors the
    remaining 896 bins.
  - Device: per core 2 batches (data-parallel). Streams weight chunks
    (128 bins x 2048 contraction) and accumulates 16 k-chunks into
    PSUM with fp32r matmuls (full PE rate at moving-dim >= 256, even).
    Frames padded 513 -> 514 and split 258+256 (fp32r needs even N,
    PSUM bank caps N at 512).
"""

import sys

sys.path.insert(0, "/opt/trn_rl_repo")

import numpy as np

BATCH = 16
LENGTH = 262144
N_FFT = 2048
HOP = 512
FRAMES = 513          # LENGTH // HOP + 1
PAD_FRAMES = 514      # frames padded to even for fp32r
M_CHUNKS = 9          # bin chunks of 128 computed on device
M_KEEP = M_CHUNKS * 128   # 1152 bins; 896 more mirrored on host
BLOCKS = 516          # padded length 264192 / 512
BT_COLS = 520         # blocks padded so shifted views stay in range
N_GROUPS = ((0, 258), (258, 256))  # frame groups: start, size (even)
CORES = 8
B_PER_CORE = BATCH // CORES

_cache = {}


def _build_device_kernel():
    import concourse.bacc as bacc
    import concourse.mybir as mybir
    from concourse import tile

    nc = bacc.Bacc("TRN2", target_bir_lowering=False, debug=False,
                   num_devices=CORES)
    f32 = mybir.dt.float32
    f32r = mybir.dt.float32r

    bt_d = nc.dram_tensor("bt", [B_PER_CORE, 4, 128, BT_COLS], f32r,
                          kind="ExternalInput")
    w_d = nc.dram_tensor("w", [2 * M_CHUNKS, 128, 16, 128], f32r,
                         kind="ExternalInput")
    o_d = nc.dram_tensor("o", [B_PER_CORE, 2 * M_CHUNKS, 128, PAD_FRAMES],
                         f32, kind="ExternalOutput")

    with tile.TileContext(nc) as tc:
        with (
            tc.tile_pool(name="btp", bufs=1) as btp,
            tc.tile_pool(name="wp", bufs=4) as wp,
            tc.tile_pool(name="op", bufs=4) as op,
            tc.tile_pool(name="psp", bufs=8, space="PSUM") as psp,
        ):
            bts = [[None] * 4 for _ in range(B_PER_CORE)]
            for b in range(B_PER_CORE):
                for c in range(4):
                    t = btp.tile([128, BT_COLS], f32r, tag=f"bt{b}{c}")
                    nc.sync.dma_start(out=t, in_=bt_d[b, c])
                    bts[b][c] = t
            for u in range(2 * M_CHUNKS):
                wt = wp.tile([128, 16, 128], f32r)
                nc.sync.dma_start(out=wt, in_=w_d[u])
                for b in range(B_PER_CORE):
                    ot = op.tile([128, PAD_FRAMES], f32)
                    for f0, ng in N_GROUPS:
                        ps = psp.tile([128, ng], f32)
                        for c in range(16):
                            rhs = bts[b][c % 4][:, f0 + c // 4:
                                                f0 + c // 4 + ng]
                            nc.tensor.matmul(ps, wt[:, c, :], rhs,
                                             start=(c == 0), stop=(c == 15))
                        nc.vector.tensor_copy(out=ot[:, f0:f0 + ng], in_=ps)
                    nc.sync.dma_start(out=o_d[b, u], in_=ot)
    nc.compile()
    return nc


def _get_nc():
    if "nc" not in _cache:
        _cache["nc"] = _build_device_kernel()
    return _cache["nc"]


def _host_prep(x, wsin, wcos):
    x = np.asarray(x, dtype=np.float32)
    wsin = np.asarray(wsin, dtype=np.float32).reshape(N_FFT, N_FFT)
    wcos = np.asarray(wcos, dtype=np.float32).reshape(N_FFT, N_FFT)

    xpad = np.pad(x, ((0, 0), (N_FFT // 2, N_FFT // 2)), mode="reflect")
    bt = np.zeros((BATCH, 4, 128, BT_COLS), np.float32)
    bt[:, :, :, :BLOCKS] = xpad.reshape(BATCH, BLOCKS, 4, 128) \
                               .transpose(0, 2, 3, 1)

    # w_host[kern*9+mc, jj, c, mm] = WT[128c+jj, 128mc+mm],
    # WT = W.T (contraction n on rows); minus folded into sin kernel.
    w_host = np.empty((2 * M_CHUNKS, 128, 16, 128), np.float32)
    for kern, wmat in enumerate((wcos, -wsin)):
        wt_full = np.ascontiguousarray(wmat[:M_KEEP].T)  # (2048, 1152)
        # (16c, 128jj, 9mc, 128mm) -> (mc, jj, c, mm)
        w_host[kern * M_CHUNKS:(kern + 1) * M_CHUNKS] = (
            wt_full.reshape(16, 128, M_CHUNKS, 128).transpose(2, 1, 0, 3))
    return bt, w_host


def _host_assemble(outs):
    # outs: list of 8 arrays (B_PER_CORE, 18, 128, 514)
    o = np.concatenate(outs, axis=0)[..., :FRAMES]  # (16, 18, 128, 513)
    o = o.reshape(BATCH, 2, M_KEEP, FRAMES)
    real_h, d_h = o[:, 0], o[:, 1]  # d = -imag
    mirror = slice(N_FFT - M_KEEP, 0, -1)  # bins 896..1
    real = np.concatenate([real_h, real_h[:, mirror]], axis=1)
    nimag = np.concatenate([d_h, -d_h[:, mirror]], axis=1)
    return (np.ascontiguousarray(real, dtype=np.float32),
            np.ascontiguousarray(nimag, dtype=np.float32))


def kernel(x, wsin, wcos):
    from concourse.bass_utils import run_bass_kernel_spmd

    nc = _get_nc()
    bt, w_host = _host_prep(x, wsin, wcos)
    in_maps = [
        {"bt": bt[i * B_PER_CORE:(i + 1) * B_PER_CORE], "w": w_host}
        for i in range(CORES)
    ]
    res = run_bass_kernel_spmd(nc, in_maps, core_ids=list(range(CORES)))
    return _host_assemble([res.results[i]["o"] for i in range(CORES)])


# revision 5
# speedup vs baseline: 1.6110x; 1.6110x over previous
"""STFT (DFT-as-conv) kernel for Trainium2, 8 NeuronCores.

Problem: x (16, 262144) f32, hann-windowed DFT kernels wsin/wcos
(2048, 1, 2048); reference reflect-pads by 1024, convolves with hop 512
-> returns (real, -imag), each (16, 2048, 513) f32.

Strategy:
  - Data-parallel over batch: 2 batches per core.
  - Hop-block im2col: n_fft = 4*hop, so the frame matrix is 4 shifted
    views of bt[b, cc, jj, m] = xpad[b, 512*m + 128*cc + jj].
  - Spectral symmetry: bins k and 2048-k mirror (cos even, sin odd);
    device computes bins 0..1151, host mirrors the remaining 896.
  - Time-reversal fold: the hann window is symmetric, so
    W[k, 2048-n] = +/- W[k, n]. Device folds frames into
    z+/-[c] = y[n] +/- y[2048-n] (DVE adds on shifted views of bt and a
    host-prepared reversed copy rev4), halving the contraction to 1024.
    win[0] = 0 kills the unpaired n=0 lane; sin(pi*n) = 0 kills the sin
    n=1024 term; the cos n=1024 term is a K=1 rank-1 matmul.
  - fp32r matmuls (full PE rate at even moving-dim >= 256). Frames
    padded 513 -> 514, split 258+256 (PSUM bank caps N at 512).
"""

import sys

sys.path.insert(0, "/opt/trn_rl_repo")

import numpy as np

BATCH = 16
LENGTH = 262144
N_FFT = 2048
HOP = 512
FRAMES = 513          # LENGTH // HOP + 1
PAD_FRAMES = 514      # frames padded to even for fp32r
M_CHUNKS = 9          # bin chunks of 128 computed on device
M_KEEP = M_CHUNKS * 128   # 1152 bins; 896 more mirrored on host
BLOCKS = 516          # padded length 264192 / 512
BT_COLS = 520         # blocks padded so shifted views stay in range
N_GROUPS = ((0, 258), (258, 256))  # frame groups: start, size (even)
CORES = 8
B_PER_CORE = BATCH // CORES
EXT = HOP * BT_COLS + 1537  # zero-extended xpad length for rev4 strides

_cache = {}


def _build_device_kernel():
    import concourse.bacc as bacc
    import concourse.mybir as mybir
    from concourse import tile

    nc = bacc.Bacc("TRN2", target_bir_lowering=False, debug=False,
                   num_devices=CORES)
    f32 = mybir.dt.float32
    f32r = mybir.dt.float32r

    bt_d = nc.dram_tensor("bt", [B_PER_CORE, 4, 128, BT_COLS], f32r,
                          kind="ExternalInput")
    rv_d = nc.dram_tensor("rv", [B_PER_CORE, 4, 128, BT_COLS], f32r,
                          kind="ExternalInput")
    w_d = nc.dram_tensor("w", [2 * M_CHUNKS, 128, 8, 128], f32r,
                         kind="ExternalInput")
    wn_d = nc.dram_tensor("wn", [1, M_CHUNKS, 128], f32r,
                          kind="ExternalInput")
    o_d = nc.dram_tensor("o", [B_PER_CORE, 2 * M_CHUNKS, 128, PAD_FRAMES],
                         f32, kind="ExternalOutput")

    with tile.TileContext(nc) as tc:
        with (
            tc.tile_pool(name="btp", bufs=1) as btp,
            tc.tile_pool(name="zp", bufs=1) as zpool,
            tc.tile_pool(name="wp", bufs=4) as wp,
            tc.tile_pool(name="op", bufs=4) as op,
            tc.tile_pool(name="psp", bufs=8, space="PSUM") as psp,
        ):
            bts = [[None] * 4 for _ in range(B_PER_CORE)]
            rvs = [[None] * 4 for _ in range(B_PER_CORE)]
            for b in range(B_PER_CORE):
                for c in range(4):
                    t = btp.tile([128, BT_COLS], f32r, tag=f"bt{b}{c}")
                    nc.sync.dma_start(out=t, in_=bt_d[b, c])
                    bts[b][c] = t
                    r = btp.tile([128, BT_COLS], f32r, tag=f"rv{b}{c}")
                    nc.sync.dma_start(out=r, in_=rv_d[b, c])
                    rvs[b][c] = r
            wnt = btp.tile([1, M_CHUNKS, 128], f32r, tag="wn")
            nc.sync.dma_start(out=wnt, in_=wn_d[0])

            # z[s][b][c]: folded frames, s=0 -> +, s=1 -> -
            zt = [[[None] * 8 for _ in range(B_PER_CORE)] for _ in range(2)]
            for b in range(B_PER_CORE):
                for c in range(8):
                    sh, rh = c // 4, 1 - c // 4
                    bv = bts[b][c % 4][:, sh:sh + PAD_FRAMES]
                    rv = rvs[b][c % 4][:, rh:rh + PAD_FRAMES]
                    for s, dve_op in ((0, nc.vector.tensor_add),
                                      (1, nc.vector.tensor_sub)):
                        z = zpool.tile([128, PAD_FRAMES], f32r,
                                       tag=f"z{s}{b}{c}")
                        dve_op(out=z, in0=bv, in1=rv)
                        zt[s][b][c] = z

            for u in range(2 * M_CHUNKS):
                kern, mc = divmod(u, M_CHUNKS)
                wt = wp.tile([128, 8, 128], f32r)
                nc.sync.dma_start(out=wt, in_=w_d[u])
                for b in range(B_PER_CORE):
                    ot = op.tile([128, PAD_FRAMES], f32)
                    for f0, ng in N_GROUPS:
                        ps = psp.tile([128, ng], f32)
                        for c in range(8):
                            nc.tensor.matmul(
                                ps, wt[:, c, :],
                                zt[kern][b][c][:, f0:f0 + ng],
                                start=(c == 0),
                                stop=(c == 7 and kern == 1))
                        if kern == 0:
                            # + wcos[:, 1024] (x) y_f[1024]  (rank-1, K=1)
                            nc.tensor.matmul(
                                ps, wnt[:, mc, :],
                                bts[b][0][0:1, f0 + 2:f0 + 2 + ng],
                                start=False, stop=True)
                        nc.vector.tensor_copy(out=ot[:, f0:f0 + ng], in_=ps)
                    nc.sync.dma_start(out=o_d[b, u], in_=ot)
    nc.compile()
    return nc


def _get_nc():
    if "nc" not in _cache:
        _cache["nc"] = _build_device_kernel()
    return _cache["nc"]


def _host_prep(x, wsin, wcos):
    x = np.asarray(x, dtype=np.float32)
    wsin = np.asarray(wsin, dtype=np.float32).reshape(N_FFT, N_FFT)
    wcos = np.asarray(wcos, dtype=np.float32).reshape(N_FFT, N_FFT)

    xpad = np.pad(x, ((0, 0), (N_FFT // 2, N_FFT // 2)), mode="reflect")
    plen = xpad.shape[1]
    bt = np.zeros((BATCH, 4, 128, BT_COLS), np.float32)
    bt[:, :, :, :BLOCKS] = xpad.reshape(BATCH, BLOCKS, 4, 128) \
                               .transpose(0, 2, 3, 1)

    # rev4[b, cc, jj, m] = xe[512m + 1536 - 128cc - jj] (zero-extended)
    xe = np.zeros((BATCH, EXT), np.float32)
    xe[:, :plen] = xpad
    swv = np.lib.stride_tricks.sliding_window_view(xe, 512, axis=1)
    q = swv[:, 1025::HOP, :][:, :BT_COLS]      # [b, m, q] = xe[512m+1025+q]
    rev4 = np.ascontiguousarray(
        q[:, :, ::-1].transpose(0, 2, 1)).reshape(BATCH, 4, 128, BT_COLS)

    # folded weights wf[kern*9+mc, jj, c, mm] = wm[128mc+mm, 128c+jj];
    # minus folded into the sin kernel (reference returns -imag).
    wf = np.empty((2 * M_CHUNKS, 128, 8, 128), np.float32)
    for kern, wm in enumerate((wcos, -wsin)):
        wk = np.ascontiguousarray(wm[:M_KEEP, :1024].T)  # (1024, 1152)
        wf[kern * M_CHUNKS:(kern + 1) * M_CHUNKS] = (
            wk.reshape(8, 128, M_CHUNKS, 128).transpose(2, 1, 0, 3))
    wn = np.ascontiguousarray(
        wcos[:M_KEEP, 1024].reshape(1, M_CHUNKS, 128))
    return bt, rev4, wf, wn


def _host_assemble(outs):
    # outs: list of 8 arrays (B_PER_CORE, 18, 128, 514)
    o = np.concatenate(outs, axis=0)[..., :FRAMES]  # (16, 18, 128, 513)
    o = o.reshape(BATCH, 2, M_KEEP, FRAMES)
    real_h, d_h = o[:, 0], o[:, 1]  # d = -imag
    mirror = slice(N_FFT - M_KEEP, 0, -1)  # bins 896..1
    real = np.concatenate([real_h, real_h[:, mirror]], axis=1)
    nimag = np.concatenate([d_h, -d_h[:, mirror]], axis=1)
    return (np.ascontiguousarray(real, dtype=np.float32),
            np.ascontiguousarray(nimag, dtype=np.float32))


def kernel(x, wsin, wcos):
    from concourse.bass_utils import run_bass_kernel_spmd

    nc = _get_nc()
    bt, rev4, wf, wn = _host_prep(x, wsin, wcos)
    in_maps = [
        {"bt": bt[i * B_PER_CORE:(i + 1) * B_PER_CORE],
         "rv": rev4[i * B_PER_CORE:(i + 1) * B_PER_CORE],
         "w": wf, "wn": wn}
        for i in range(CORES)
    ]
    res = run_bass_kernel_spmd(nc, in_maps, core_ids=list(range(CORES)))
    return _host_assemble([res.results[i]["o"] for i in range(CORES)])


# revision 11
# speedup vs baseline: 1.6726x; 1.0382x over previous
"""STFT (DFT-as-conv) kernel for Trainium2, 8 NeuronCores.

Problem: x (16, 262144) f32, hann-windowed DFT kernels wsin/wcos
(2048, 1, 2048); reference reflect-pads by 1024, convolves with hop 512
-> returns (real, -imag), each (16, 2048, 513) f32.

Strategy:
  - Data-parallel over batch: 2 batches per core.
  - Hop-block im2col: n_fft = 4*hop, so the frame matrix is 4 shifted
    views of bt[b, cc, jj, m] = xpad[b, 512*m + 128*cc + jj].
  - Spectral symmetry: bins k and 2048-k mirror (cos even, sin odd);
    device computes bins 0..1151, host mirrors the remaining 896.
  - Time-reversal fold: the hann window is symmetric, so
    W[k, 2048-n] = +/- W[k, n]. Device folds frames into
    z+/-[c] = y[n] +/- y[2048-n] (DVE adds on shifted views of bt and a
    host-prepared reversed copy rev4), halving the contraction to 1024.
    win[0] = 0 kills the unpaired n=0 lane; sin(pi*n) = 0 kills the sin
    n=1024 term; the cos n=1024 term is a K=1 rank-1 matmul.
  - fp32r matmuls (full PE rate at even moving-dim >= 256). Frames
    padded 513 -> 514, split 258+256 (PSUM bank caps N at 512).
"""

import sys

sys.path.insert(0, "/opt/trn_rl_repo")

import numpy as np

BATCH = 16
LENGTH = 262144
N_FFT = 2048
HOP = 512
FRAMES = 513          # LENGTH // HOP + 1
PAD_FRAMES = 514      # frames padded to even for fp32r
M_CHUNKS = 9          # bin chunks of 128 computed on device
M_KEEP = M_CHUNKS * 128   # 1152 bins; 896 more mirrored on host
BLOCKS = 516          # padded length 264192 / 512
BT_COLS = 520         # blocks padded so shifted views stay in range
N_GROUPS = ((0, 258), (258, 256))  # frame groups: start, size (even)
CORES = 8
B_PER_CORE = BATCH // CORES
EXT = HOP * BT_COLS + 1537  # zero-extended xpad length for rev4 strides

_cache = {}


def _build_device_kernel():
    import concourse.bacc as bacc
    import concourse.mybir as mybir
    from concourse import tile

    nc = bacc.Bacc("TRN2", target_bir_lowering=False, debug=False,
                   num_devices=CORES)
    f32 = mybir.dt.float32
    f32r = mybir.dt.float32r

    bt_d = nc.dram_tensor("bt", [B_PER_CORE, 4, 128, BT_COLS], f32r,
                          kind="ExternalInput")
    rv_d = nc.dram_tensor("rv", [B_PER_CORE, 4, 128, BT_COLS], f32r,
                          kind="ExternalInput")
    w_d = nc.dram_tensor("w", [2 * M_CHUNKS, 128, 8, 128], f32r,
                         kind="ExternalInput")
    o_d = nc.dram_tensor("o", [B_PER_CORE, 2 * M_CHUNKS, 128, PAD_FRAMES],
                         f32, kind="ExternalOutput")

    with tile.TileContext(nc) as tc:
        with (
            tc.tile_pool(name="btp", bufs=1) as btp,
            tc.tile_pool(name="zp", bufs=1) as zpool,
            tc.tile_pool(name="wp", bufs=4) as wp,
            tc.tile_pool(name="op", bufs=4) as op,
            tc.tile_pool(name="psp", bufs=8, space="PSUM") as psp,
        ):
            bts = [[None] * 4 for _ in range(B_PER_CORE)]
            rvs = [[None] * 4 for _ in range(B_PER_CORE)]
            for b in range(B_PER_CORE):
                for c in range(4):
                    t = btp.tile([128, BT_COLS], f32r, tag=f"bt{b}{c}")
                    nc.sync.dma_start(out=t, in_=bt_d[b, c])
                    bts[b][c] = t
                    r = btp.tile([128, BT_COLS], f32r, tag=f"rv{b}{c}")
                    nc.sync.dma_start(out=r, in_=rv_d[b, c])
                    rvs[b][c] = r
            # z[s][b][c]: folded frames, s=0 -> +, s=1 -> -
            zt = [[[None] * 8 for _ in range(B_PER_CORE)] for _ in range(2)]
            for b in range(B_PER_CORE):
                for c in range(8):
                    sh, rh = c // 4, 1 - c // 4
                    bv = bts[b][c % 4][:, sh:sh + PAD_FRAMES]
                    rv = rvs[b][c % 4][:, rh:rh + PAD_FRAMES]
                    for s, dve_op in ((0, nc.vector.tensor_add),
                                      (1, nc.vector.tensor_sub)):
                        z = zpool.tile([128, PAD_FRAMES], f32r,
                                       tag=f"z{s}{b}{c}")
                        dve_op(out=z, in0=bv, in1=rv)
                        zt[s][b][c] = z
                # lane (c=0, jj=0) carries the n=1024 cos term: win[0] = 0
                # frees the n=0 weight slot, so host puts wcos[:, 1024]
                # there and z+ lane 0 must hold y_f[1024].
                nc.vector.tensor_copy(
                    out=zt[0][b][0][0:1, :],
                    in_=bts[b][0][0:1, 2:2 + PAD_FRAMES])

            for u in range(2 * M_CHUNKS):
                kern, mc = divmod(u, M_CHUNKS)
                wt = wp.tile([128, 8, 128], f32r)
                nc.sync.dma_start(out=wt, in_=w_d[u])
                for b in range(B_PER_CORE):
                    ot = op.tile([128, PAD_FRAMES], f32)
                    for f0, ng in N_GROUPS:
                        ps = psp.tile([128, ng], f32)
                        for c in range(8):
                            nc.tensor.matmul(
                                ps, wt[:, c, :],
                                zt[kern][b][c][:, f0:f0 + ng],
                                start=(c == 0), stop=(c == 7))
                        nc.vector.tensor_copy(out=ot[:, f0:f0 + ng], in_=ps)
                    nc.sync.dma_start(out=o_d[b, u], in_=ot)
    nc.compile()
    return nc


def _get_nc():
    if "nc" not in _cache:
        _cache["nc"] = _build_device_kernel()
    return _cache["nc"]


def _host_prep(x, wsin, wcos):
    x = np.asarray(x, dtype=np.float32)
    wsin = np.asarray(wsin, dtype=np.float32).reshape(N_FFT, N_FFT)
    wcos = np.asarray(wcos, dtype=np.float32).reshape(N_FFT, N_FFT)

    xpad = np.pad(x, ((0, 0), (N_FFT // 2, N_FFT // 2)), mode="reflect")
    plen = xpad.shape[1]
    bt = np.zeros((BATCH, 4, 128, BT_COLS), np.float32)
    bt[:, :, :, :BLOCKS] = xpad.reshape(BATCH, BLOCKS, 4, 128) \
                               .transpose(0, 2, 3, 1)

    # rev4[b, cc, jj, m] = xe[512m + 1536 - 128cc - jj] (zero-extended)
    xe = np.zeros((BATCH, EXT), np.float32)
    xe[:, :plen] = xpad
    swv = np.lib.stride_tricks.sliding_window_view(xe, 512, axis=1)
    q = swv[:, 1025::HOP, :][:, :BT_COLS]      # [b, m, q] = xe[512m+1025+q]
    rev4 = np.ascontiguousarray(
        q[:, :, ::-1].transpose(0, 2, 1)).reshape(BATCH, 4, 128, BT_COLS)

    # folded weights wf[kern*9+mc, jj, c, mm] = wm[128mc+mm, 128c+jj];
    # minus folded into the sin kernel (reference returns -imag).
    wf = np.empty((2 * M_CHUNKS, 128, 8, 128), np.float32)
    for kern, wm in enumerate((wcos, -wsin)):
        wk = np.ascontiguousarray(wm[:M_KEEP, :1024].T)  # (1024, 1152)
        wf[kern * M_CHUNKS:(kern + 1) * M_CHUNKS] = (
            wk.reshape(8, 128, M_CHUNKS, 128).transpose(2, 1, 0, 3))
    # n=0 lane is dead (win[0] = 0); reuse it for the cos n=1024 column
    # (z+ chunk-0 lane 0 is patched to y_f[1024] on device).
    wf[:M_CHUNKS, 0, 0, :] = wcos[:M_KEEP, 1024].reshape(M_CHUNKS, 128)
    return bt, rev4, wf


def _host_assemble(outs):
    # outs: list of 8 arrays (B_PER_CORE, 18, 128, 514)
    o = np.concatenate(outs, axis=0)[..., :FRAMES]  # (16, 18, 128, 513)
    o = o.reshape(BATCH, 2, M_KEEP, FRAMES)
    real_h, d_h = o[:, 0], o[:, 1]  # d = -imag
    mirror = slice(N_FFT - M_KEEP, 0, -1)  # bins 896..1
    real = np.concatenate([real_h, real_h[:, mirror]], axis=1)
    nimag = np.concatenate([d_h, -d_h[:, mirror]], axis=1)
    return (np.ascontiguousarray(real, dtype=np.float32),
            np.ascontiguousarray(nimag, dtype=np.float32))


def kernel(x, wsin, wcos):
    from concourse.bass_utils import run_bass_kernel_spmd

    nc = _get_nc()
    bt, rev4, wf = _host_prep(x, wsin, wcos)
    in_maps = [
        {"bt": bt[i * B_PER_CORE:(i + 1) * B_PER_CORE],
         "rv": rev4[i * B_PER_CORE:(i + 1) * B_PER_CORE],
         "w": wf}
        for i in range(CORES)
    ]
    res = run_bass_kernel_spmd(nc, in_maps, core_ids=list(range(CORES)))
    return _host_assemble([res.results[i]["o"] for i in range(CORES)])
